# revision 17
# baseline (speedup 1.0000x reference)
"""Bass/Tile TRN2 kernel for the MeanFieldGaussianLayer loss.

reference math:
    mean  = tensor[:, :, 0]                       (B, T)
    f_var = softplus(tensor[:, :, 1])
    y_var = f_var + softplus(noise) + 1e-6
    logp  = -0.5 * sum_T(LOG_2PI + log(y_var) + (y - mean)^2 / y_var)
    out   = mean_B(logp)

Strategy: pure data-parallel over B across 8 cores; 64 rows/core as
[128, 8192] planes, staged to the device in fp16 (inputs are ~N(0,1);
fp16 quantization moves the result by ~1e-4 rel, far inside the 2e-2
gate, and halves HBM traffic).

Per-core pipeline (a = exp(softplus(noise) + 1e-6), c = ln a):
    ACT:  u = Exp(t1) (fp32); v = Ln(a*u + a) = softplus(t1)+c (fp16)
    DVE:  d = y - t0 (fp16 stock, 2x rate); fused SQDIV_RED:
          accum += d^2 * recip1(v)      (1-NR bit-trick reciprocal)
    Σ log(v), split to balance ACT and DVE finishing times:
      [0, RED_END):   Mitchell bit-trick on DVE — stock int16-view
          reduce of v's fp16 bit patterns ≈ 1024*(log2 v + 15 - σ)
      [RED_END, end): one exact Ln+accumulate ACT instruction, placed
          at the very end of the scalar stream so the DVE drains its
          sqdiv backlog underneath it.
Final: per-partition partials -> PE ones-matmul folds 128 partitions,
single-descriptor [1,3] DMA out; host undoes the Mitchell affine map
and combines cores in fp64.

All input tiles are fresh (DMA never waits on SBUF reuse), every input
descriptor is generated up front on sync in exact FIFO order (t1 leads:
it gates the serial ACT chain, whose ~22us is the critical path), and
the first t1 transfer is small so its slowest queue-completion — the
ACT start gate — lands early.  GpSimd is left idle on purpose: any
heavy GpSimd op running concurrently with DVE work slows both ~2.5x
(SBUF port contention, measured).
"""

import os
import sys

import numpy as np

if "/opt/trn_rl_repo" not in sys.path:
    sys.path.insert(0, "/opt/trn_rl_repo")

import concourse.bass as bass
import concourse.tile as tile
from concourse import bacc, mybir
from concourse import bass_utils

# ---------------------------------------------------------------------------
# Patch 1: force all ACT functions into the one table set that contains
# Exp+Ln. bacc's insert_act_table_loads otherwise flip-flops between
# `exp_and_others` and `natural_log` (first-match), costing a ~1.3us
# ACT_TABLE_LOAD per switch.
# ---------------------------------------------------------------------------
import concourse.bacc as _bacc_mod

_ACT_KEEP = "natural_log_exp_and_others"
_ACT_STRIP = {
    mybir.ActivationFunctionType.Exp,
    mybir.ActivationFunctionType.Ln,
    mybir.ActivationFunctionType.Square,
}
_orig_get_tables = _bacc_mod.get_activation_tables


def _patched_get_tables(arch):
    # Empty every other table so first-match can only ever pick the keep
    # table; table ids (= position in this dict) must stay unchanged.
    tabs = _orig_get_tables(arch)
    return {
        name: (set(fns) if name == _ACT_KEEP else set())
        for name, fns in tabs.items()
    }


_bacc_mod.get_activation_tables = _patched_get_tables

# ---------------------------------------------------------------------------
# Patch 1b: cheaper Tile kernel tail (drop the trailing all-engine barrier;
# keep drain + first barrier + sem/DMA clears for re-execution safety).
# ---------------------------------------------------------------------------
import concourse.tile as _tile_mod
from concourse.vector_clock import ScopedClock as _ScopedClock


def _cheap_drain_and_barrier(self, tick_clock, wait_clock):
    drain_inst = self.nc.sync.drain()
    wait_clock.add_sem_waits(
        drain_inst.ins, _ScopedClock({None: tick_clock.global_clock})
    )
    self.nc.all_engine_barrier()
    popped = self.nc._tile_sem_poison_stack.pop()
    assert popped is self._sem_poison
    self.nc.clear_and_free_semaphores(list(self.sems.allocated().values()))


_tile_mod.TileContext._drain_and_barrier = _cheap_drain_and_barrier

# ---------------------------------------------------------------------------
# Patch 2: custom DVE op SQDIV_RED_ANT:
#   out = Src0^2 * recip1(Src1) ; accum_out = sum(out)
# recip1 = one-NR bit-trick reciprocal (max rel err ~0.18%).
# ---------------------------------------------------------------------------
import concourse.dve_ops as _dve_ops
from concourse.dve_ops import DveOp, _ref_body_sum, RECIP_APPROX_FAST_CONSTS
from concourse.dve_spec import (
    C0,
    C1,
    Spec,
    Src0,
    Src1,
    _has_src1,
    lower,
    sq,
    AluOp,
)
from concourse.dve_spec import Bin as _Bin
from concourse.dve_uop import DveOpSpec
from operator import add as _op_add

RC0 = float(RECIP_APPROX_FAST_CONSTS["s0"])
RC1 = float(RECIP_APPROX_FAST_CONSTS["s1"])


def _register_custom(name, spec):
    if name in _dve_ops._SUB_OPCODE_FOR_NAME:
        return next(op for op in _dve_ops.OPS if op.name == name)
    row = max(_dve_ops._SUB_OPCODE_FOR_NAME.values()) + 1
    assert row < 0x20
    shas = {}
    for ver in ("v3", "v4"):
        try:
            uops = lower(spec, ver=ver)
            shas[ver] = DveOpSpec(
                name=name, opcode=row, uops=uops, rd1_en=_has_src1(spec)
            ).sha(ver)
        except Exception:
            pass
    assert shas, f"lower() failed for {name} on all vers"
    op = DveOp(name, spec, subdim=False, uops_sha=shas)
    _dve_ops._SUB_OPCODE_FOR_NAME[name] = row
    _dve_ops.OPS.append(op)
    _dve_ops.CUSTOM_DVE_SPECS[name] = spec
    return op


def _make_sqdiv_red():
    _not_v = _Bin(AluOp.BITWISE_NOT, Src1, Src1)
    _y0 = _not_v * C0
    _y1 = _y0 * (C1 - Src1 * _y0)
    body = sq(Src0) * _y1

    def _ref(in0, in1, c0, c1, c2):
        not_v = (~in1.astype(np.float32).view(np.int32)).view(np.float32)
        y0 = not_v * c0
        y1 = y0 * (c1 - in1 * y0)
        return np.square(in0.astype(np.float32)) * y1

    spec = Spec(
        body=body,
        accum=_op_add,
        accum_init=None,
        reference=_ref_body_sum(_ref),
    )
    return _register_custom("SQDIV_RED_ANT", spec)


SQDIV_RED = _make_sqdiv_red()

B, T = 512, 16384
NCORES = 8
ROWS = B // NCORES          # 64 rows per core
P = 128                     # SBUF partitions
FPP = ROWS * T // P         # 8192 elements per partition per plane

# Unified chunk/grain schedule: ACT grains (exp+ln) and yt0 chunks are
# 1:1 so each chunk's DVE work gates on exactly its own ln.  Small first
# grain starts the ACT->DVE chain early; big middle grains amortize the
# ~170ns per-instruction ACT overhead; small final grains shorten the
# tail.
FDS = [512, 1024, 2048, 2048, 2048, 512]
assert sum(FDS) == FPP
NT = len(FDS)
# t1 DMA transfers (into one big tile; subtile deps gate ACT grains):
# tiny first transfer so its slowest queue-completion (the ACT chain's
# start gate) lands ~3us earlier; fat later for DMA line efficiency
TDS = [512, 1024, 2048, 4608]
assert sum(TDS) == FPP
# GpSimd is left COMPLETELY idle: any heavy GpSimd op running while the
# DVE works slows both ~2.5x (SBUF port contention, measured).
# Sum(log v) split at column RED_END: [0, RED_END) via the Mitchell
# bit-sum on DVE (slices below, emitted as their ln grains land);
# [RED_END, FPP) via one exact Ln+accumulate tail instruction on ACT.
# Balances the two engines' finishing times.
RED_END = 4608
RED_SLICES = [(0, 512), (512, 1536), (1536, 3584), (3584, 4608)]

LOG_2PI = float(np.log(2.0 * np.pi))
JITTER = 1e-6
MITCHELL_SIGMA = 1.5 - 1.0 / float(np.log(2.0))   # E[log2(1+m) - m], m~U[0,1)
LN2 = float(np.log(2.0))

_BUILD_CACHE: dict[float, object] = {}
LAST_RESULT = None


def _strip_boot_preamble(nc):
    """Drop the const-AP memsets + boot all-engine barrier from `main`.
    Nothing here reads the const APs (all activation biases are explicit
    SBUF tiles); the preamble only adds ~1us of serial boot latency."""
    main_bb = nc.m.functions[0].blocks[0]
    assert main_bb.name == "main"
    drop = (mybir.InstMemset, mybir.InstDrain, mybir.InstEventSemaphore)
    main_bb.instructions[:] = [
        i for i in main_bb.instructions if not isinstance(i, drop)
    ]


def _build(a: float):
    f32 = mybir.dt.float32
    f16 = mybir.dt.float16
    i16 = mybir.dt.int16
    Act = mybir.ActivationFunctionType

    nc = bacc.Bacc("TRN2", target_bir_lowering=False, debug=False)
    _strip_boot_preamble(nc)

    t1 = nc.dram_tensor("t1", [P, FPP], f16, kind="ExternalInput").ap()
    yt0 = nc.dram_tensor("yt0", [P, 2 * FPP], f16, kind="ExternalInput").ap()
    out = nc.dram_tensor("out", [1, 3], f32, kind="ExternalOutput").ap()

    offs = [0]
    for FD in FDS:
        offs.append(offs[-1] + FD)
    toffs = [0]
    for FD in TDS:
        toffs.append(toffs[-1] + FD)

    with tile.TileContext(nc) as tc:
        with (
            tc.tile_pool(name="io", bufs=1) as io,
            tc.tile_pool(name="mid", bufs=2) as mid,
            tc.tile_pool(name="accs", bufs=1) as accs,
            tc.tile_pool(name="psum", bufs=1, space=bass.MemorySpace.PSUM) as psum,
        ):
            acc_p = accs.tile([P, NT], f32)   # per-chunk sum(d^2/v)
            acc_r = accs.tile([P, len(RED_SLICES)], f32)  # bit-sum partials
            acc_l = accs.tile([P, 1], f32)    # exact ln partial (tail range)
            outt = accs.tile([P, 3], f32)
            outs = accs.tile([1, 3], f32)
            pacc = psum.tile([1, 3], f32)
            abias = accs.tile([P, 1], f32)
            zbias = accs.tile([P, 1], f32)
            ones = accs.tile([P, 1], f32)
            nc.vector.memset(abias[:], a)
            nc.vector.memset(zbias[:], 0.0)
            nc.vector.memset(ones[:], 1.0)

            # big contiguous t1 / u / v planes; DMA + ACT use subtile ranges
            t1t = io.tile([P, FPP], f16, tag="t1t")
            ut = io.tile([P, FPP], f32, tag="ut")
            vt = io.tile([P, FPP], f16, tag="vt")
            vt_i16 = vt.bitcast(i16)

            yt0_t = {}

            def dma_t1(eng, k):
                eng.dma_start(
                    t1t[:, toffs[k] : toffs[k + 1]], t1[:, toffs[k] : toffs[k + 1]]
                )

            def dma_yt0(eng, i):
                yt0_t[i] = io.tile(
                    [P, 2 * FDS[i]], f16, tag=f"yt0_{i}", name=f"tyt0_{i}"
                )
                eng.dma_start(yt0_t[i][:], yt0[:, 2 * offs[i] : 2 * offs[i + 1]])

            # All descriptors on one engine (sync): program order == queue
            # FIFO order == arrival order, no cross-engine races.  t1 leads
            # (it gates the serial ACT chain), yt0 interleaves behind.
            dma_t1(nc.sync, 0)
            dma_t1(nc.sync, 1)
            dma_yt0(nc.sync, 0)
            dma_t1(nc.sync, 2)
            dma_t1(nc.sync, 3)
            for i in range(1, NT):
                dma_yt0(nc.sync, i)

            # Tiny dependency-free first activation: the one-time
            # ACT_TABLE_LOAD bacc inserts before the first ACT instruction
            # executes at boot instead of after t1_0 lands.
            warm = accs.tile([P, 1], f32)
            nc.scalar.activation(warm[:], zbias[:], Act.Exp, bias=zbias[:, 0:1])

            # Per-grain pipeline: ACT exp+ln; on the DVE the sub for chunk
            # i+1 is emitted BEFORE sqdiv_i, so subs execute in the DVE's
            # wait-for-ln gaps (their yt0 data lands well ahead) instead of
            # stacking up behind the sqdiv chain at the end.
            red_next = 0
            d_t = {}

            def emit_sub(j):
                tyt = yt0_t.pop(j)
                d_t[j] = mid.tile([P, FDS[j]], f16, tag="d", bufs=3, name=f"d_{j}")
                nc.vector.tensor_sub(
                    d_t[j][:], tyt[:, 0 : FDS[j]], tyt[:, FDS[j] : 2 * FDS[j]]
                )

            emit_sub(0)
            for i in range(NT):
                FD = FDS[i]
                sl = slice(offs[i], offs[i + 1])

                nc.scalar.activation(ut[:, sl], t1t[:, sl], Act.Exp, bias=zbias[:, 0:1])
                nc.scalar.activation(
                    vt[:, sl], ut[:, sl], Act.Ln, bias=abias[:, 0:1], scale=a
                )
                if i + 1 < NT:
                    emit_sub(i + 1)
                scr = mid.tile([P, FD], f16, tag="scr")
                nc.vector._custom_dve(
                    SQDIV_RED,
                    out=scr[:],
                    in0=d_t.pop(i)[:],
                    in1=vt[:, sl],
                    s0=RC0,
                    s1=RC1,
                    accum_out=acc_p[:, i : i + 1],
                )
                while red_next < len(RED_SLICES) and RED_SLICES[red_next][1] <= offs[i + 1]:
                    lo, hi = RED_SLICES[red_next]
                    nc.vector.reduce_sum(
                        acc_r[:, red_next : red_next + 1],
                        vt_i16[:, lo:hi],
                        axis=mybir.AxisListType.X,
                    )
                    red_next += 1

            # Exact sum(ln v) over the tail range as ONE ACT instruction at
            # the very end of the scalar stream: the DVE drains its
            # remaining sqdiv/bit-sum backlog underneath it, so the two
            # engines finish together instead of DVE trailing by ~3us.
            lsl = slice(RED_END, FPP)
            nc.scalar.activation(
                ut[:, lsl],
                vt[:, lsl],
                Act.Ln,
                bias=zbias[:, 0:1],
                accum_out=acc_l[:, 0:1],
            )

            # final folds: [sum d^2/v, bit-sum, exact-ln sum] per partition,
            # PE ones-matmul folds the 128 partitions -> [1, 3]
            nc.vector.reduce_sum(outt[:, 0:1], acc_p[:], axis=mybir.AxisListType.X)
            nc.vector.reduce_sum(outt[:, 1:2], acc_r[:], axis=mybir.AxisListType.X)
            nc.vector.tensor_copy(outt[:, 2:3], acc_l[:, 0:1])
            nc.tensor.matmul(pacc[:, :], ones[:, 0:1], outt[:, 0:3])
            nc.vector.tensor_copy(outs[:, :], pacc[:, :])
            nc.sync.dma_start(out[:], outs[:])

    nc.compile()
    return nc


def kernel(tensor, y_target, noise_unconstrained):
    global LAST_RESULT
    noise = np.float64(np.asarray(noise_unconstrained))
    c = np.log1p(np.exp(-abs(noise))) + max(noise, 0.0) + JITTER  # softplus+jitter
    a = float(np.exp(c))

    nc = _BUILD_CACHE.get(a)
    if nc is None:
        nc = _build(a)
        _BUILD_CACHE[a] = nc

    tensor = np.asarray(tensor, dtype=np.float32)
    y_target = np.asarray(y_target, dtype=np.float32)

    offs = [0]
    for FD in FDS:
        offs.append(offs[-1] + FD)

    in_maps = []
    for k in range(NCORES):
        sh = tensor[k * ROWS : (k + 1) * ROWS]          # (64, 16384, 2)
        t1p = sh[:, :, 1].reshape(P, FPP).astype(np.float16)
        yp = y_target[k * ROWS : (k + 1) * ROWS, :, 0].reshape(P, FPP).astype(
            np.float16
        )
        t0p = sh[:, :, 0].reshape(P, FPP).astype(np.float16)
        yt0p = np.empty((P, 2 * FPP), dtype=np.float16)
        for i in range(NT):
            lo, hi = offs[i], offs[i + 1]
            yt0p[:, 2 * lo : lo + hi] = yp[:, lo:hi]
            yt0p[:, lo + hi : 2 * hi] = t0p[:, lo:hi]
        in_maps.append({"t1": t1p, "yt0": yt0p})

    trace = os.environ.get("BASS_KERNEL_PROFILE", "0") == "1"
    res = bass_utils.run_bass_kernel_spmd(
        nc, in_maps, list(range(NCORES)), trace=trace
    )
    LAST_RESULT = res

    s_p = np.float64(0.0)    # sum d^2 / y_var
    s_i = np.float64(0.0)    # sum of fp16 bit patterns of v (early grains)
    s_ln = np.float64(0.0)   # exact sum ln v (late grains)
    for k in range(NCORES):
        o = np.asarray(res.results[k]["out"], dtype=np.float64)
        s_p += o[0, 0]
        s_i += o[0, 1]
        s_ln += o[0, 2]
    offs = [0]
    for FD in FDS:
        offs.append(offs[-1] + FD)
    n_bits = np.float64(RED_END * P * NCORES)
    n_tot = np.float64(B) * np.float64(T)
    s_ln += LN2 * (s_i / 1024.0 + n_bits * (MITCHELL_SIGMA - 15.0))
    total = n_tot * np.float64(LOG_2PI) + s_ln + s_p
    return np.array(-0.5 * total / B, dtype=np.float32)


# revision 18
# speedup vs baseline: 1.0256x; 1.0256x over previous
"""Bass/Tile TRN2 kernel for the MeanFieldGaussianLayer loss.

reference math:
    mean  = tensor[:, :, 0]                       (B, T)
    f_var = softplus(tensor[:, :, 1])
    y_var = f_var + softplus(noise) + 1e-6
    logp  = -0.5 * sum_T(LOG_2PI + log(y_var) + (y - mean)^2 / y_var)
    out   = mean_B(logp)

Strategy: pure data-parallel over B across 8 cores; 64 rows/core as
[128, 8192] planes, staged to the device in fp16 (inputs are ~N(0,1);
fp16 quantization moves the result by ~1e-4 rel, far inside the 2e-2
gate, and halves HBM traffic).

Per-core pipeline (a = exp(softplus(noise) + 1e-6), c = ln a):
    ACT:  u = Exp(t1) (fp32); v = Ln(a*u + a) = softplus(t1)+c (fp16)
    DVE:  d = y - t0 (fp16 stock, 2x rate); fused SQDIV_RED:
          accum += d^2 * recip1(v)      (1-NR bit-trick reciprocal)
    Σ log(v), split to balance ACT and DVE finishing times:
      [0, RED_END):   Mitchell bit-trick on DVE — stock int16-view
          reduce of v's fp16 bit patterns ≈ 1024*(log2 v + 15 - σ)
      [RED_END, end): one exact Ln+accumulate ACT instruction, placed
          at the very end of the scalar stream so the DVE drains its
          sqdiv backlog underneath it.
Final: per-partition partials -> PE ones-matmul folds 128 partitions,
single-descriptor [1,3] DMA out; host undoes the Mitchell affine map
and combines cores in fp64.

All input tiles are fresh (DMA never waits on SBUF reuse), every input
descriptor is generated up front on sync in exact FIFO order (t1 leads:
it gates the serial ACT chain, whose ~22us is the critical path), and
the first t1 transfer is small so its slowest queue-completion — the
ACT start gate — lands early.  GpSimd is left idle on purpose: any
heavy GpSimd op running concurrently with DVE work slows both ~2.5x
(SBUF port contention, measured).
"""

import os
import sys

import numpy as np

if "/opt/trn_rl_repo" not in sys.path:
    sys.path.insert(0, "/opt/trn_rl_repo")

import concourse.bass as bass
import concourse.tile as tile
from concourse import bacc, mybir
from concourse import bass_utils

# ---------------------------------------------------------------------------
# Patch 1: force all ACT functions into the one table set that contains
# Exp+Ln. bacc's insert_act_table_loads otherwise flip-flops between
# `exp_and_others` and `natural_log` (first-match), costing a ~1.3us
# ACT_TABLE_LOAD per switch.
# ---------------------------------------------------------------------------
import concourse.bacc as _bacc_mod

_ACT_KEEP = "natural_log_exp_and_others"
_ACT_STRIP = {
    mybir.ActivationFunctionType.Exp,
    mybir.ActivationFunctionType.Ln,
    mybir.ActivationFunctionType.Square,
}
_orig_get_tables = _bacc_mod.get_activation_tables


def _patched_get_tables(arch):
    # Empty every other table so first-match can only ever pick the keep
    # table; table ids (= position in this dict) must stay unchanged.
    tabs = _orig_get_tables(arch)
    return {
        name: (set(fns) if name == _ACT_KEEP else set())
        for name, fns in tabs.items()
    }


_bacc_mod.get_activation_tables = _patched_get_tables

# ---------------------------------------------------------------------------
# Patch 1b: cheaper Tile kernel tail (drop the trailing all-engine barrier;
# keep drain + first barrier + sem/DMA clears for re-execution safety).
# ---------------------------------------------------------------------------
import concourse.tile as _tile_mod
from concourse.vector_clock import ScopedClock as _ScopedClock


def _cheap_drain_and_barrier(self, tick_clock, wait_clock):
    drain_inst = self.nc.sync.drain()
    wait_clock.add_sem_waits(
        drain_inst.ins, _ScopedClock({None: tick_clock.global_clock})
    )
    self.nc.all_engine_barrier()
    popped = self.nc._tile_sem_poison_stack.pop()
    assert popped is self._sem_poison
    self.nc.clear_and_free_semaphores(list(self.sems.allocated().values()))


_tile_mod.TileContext._drain_and_barrier = _cheap_drain_and_barrier

# ---------------------------------------------------------------------------
# Patch 2: custom DVE op SQDIV_RED_ANT:
#   out = Src0^2 * recip1(Src1) ; accum_out = sum(out)
# recip1 = one-NR bit-trick reciprocal (max rel err ~0.18%).
# ---------------------------------------------------------------------------
import concourse.dve_ops as _dve_ops
from concourse.dve_ops import DveOp, _ref_body_sum, RECIP_APPROX_FAST_CONSTS
from concourse.dve_spec import (
    C0,
    C1,
    Spec,
    Src0,
    Src1,
    _has_src1,
    lower,
    sq,
    AluOp,
)
from concourse.dve_spec import Bin as _Bin
from concourse.dve_uop import DveOpSpec
from operator import add as _op_add

RC0 = float(RECIP_APPROX_FAST_CONSTS["s0"])
RC1 = float(RECIP_APPROX_FAST_CONSTS["s1"])


def _register_custom(name, spec):
    if name in _dve_ops._SUB_OPCODE_FOR_NAME:
        return next(op for op in _dve_ops.OPS if op.name == name)
    row = max(_dve_ops._SUB_OPCODE_FOR_NAME.values()) + 1
    assert row < 0x20
    shas = {}
    for ver in ("v3", "v4"):
        try:
            uops = lower(spec, ver=ver)
            shas[ver] = DveOpSpec(
                name=name, opcode=row, uops=uops, rd1_en=_has_src1(spec)
            ).sha(ver)
        except Exception:
            pass
    assert shas, f"lower() failed for {name} on all vers"
    op = DveOp(name, spec, subdim=False, uops_sha=shas)
    _dve_ops._SUB_OPCODE_FOR_NAME[name] = row
    _dve_ops.OPS.append(op)
    _dve_ops.CUSTOM_DVE_SPECS[name] = spec
    return op


def _make_sqdiv_red():
    _not_v = _Bin(AluOp.BITWISE_NOT, Src1, Src1)
    _y0 = _not_v * C0
    _y1 = _y0 * (C1 - Src1 * _y0)
    body = sq(Src0) * _y1

    def _ref(in0, in1, c0, c1, c2):
        not_v = (~in1.astype(np.float32).view(np.int32)).view(np.float32)
        y0 = not_v * c0
        y1 = y0 * (c1 - in1 * y0)
        return np.square(in0.astype(np.float32)) * y1

    spec = Spec(
        body=body,
        accum=_op_add,
        accum_init=None,
        reference=_ref_body_sum(_ref),
    )
    return _register_custom("SQDIV_RED_ANT", spec)


SQDIV_RED = _make_sqdiv_red()

B, T = 512, 16384
NCORES = 8
ROWS = B // NCORES          # 64 rows per core
P = 128                     # SBUF partitions
FPP = ROWS * T // P         # 8192 elements per partition per plane

# Unified chunk/grain schedule: ACT grains (exp+ln) and yt0 chunks are
# 1:1 so each chunk's DVE work gates on exactly its own ln.  Small first
# grain starts the ACT->DVE chain early; big middle grains amortize the
# ~170ns per-instruction ACT overhead; small final grains shorten the
# tail.
FDS = [512, 1024, 2048, 2048, 1536, 512, 256, 256]
assert sum(FDS) == FPP
NT = len(FDS)
# t1 DMA transfers (into one big tile; subtile deps gate ACT grains):
# tiny first transfer so its slowest queue-completion (the ACT chain's
# start gate) lands ~3us earlier; fat later for DMA line efficiency
TDS = [512, 1024, 2048, 4608]
assert sum(TDS) == FPP
# GpSimd is left COMPLETELY idle: any heavy GpSimd op running while the
# DVE works slows both ~2.5x (SBUF port contention, measured).
# Sum(log v) split at column RED_END: [0, RED_END) via the Mitchell
# bit-sum on DVE (slices below, emitted as their ln grains land);
# [RED_END, FPP) via one exact Ln+accumulate tail instruction on ACT.
# Balances the two engines' finishing times.
RED_END = 4608
RED_SLICES = [(0, 512), (512, 1536), (1536, 3584), (3584, 4608)]

LOG_2PI = float(np.log(2.0 * np.pi))
JITTER = 1e-6
MITCHELL_SIGMA = 1.5 - 1.0 / float(np.log(2.0))   # E[log2(1+m) - m], m~U[0,1)
LN2 = float(np.log(2.0))

_BUILD_CACHE: dict[float, object] = {}
LAST_RESULT = None


def _strip_boot_preamble(nc):
    """Drop the const-AP memsets + boot all-engine barrier from `main`.
    Nothing here reads the const APs (all activation biases are explicit
    SBUF tiles); the preamble only adds ~1us of serial boot latency."""
    main_bb = nc.m.functions[0].blocks[0]
    assert main_bb.name == "main"
    drop = (mybir.InstMemset, mybir.InstDrain, mybir.InstEventSemaphore)
    main_bb.instructions[:] = [
        i for i in main_bb.instructions if not isinstance(i, drop)
    ]


def _build(a: float):
    f32 = mybir.dt.float32
    f16 = mybir.dt.float16
    i16 = mybir.dt.int16
    Act = mybir.ActivationFunctionType

    nc = bacc.Bacc("TRN2", target_bir_lowering=False, debug=False)
    _strip_boot_preamble(nc)

    t1 = nc.dram_tensor("t1", [P, FPP], f16, kind="ExternalInput").ap()
    yt0 = nc.dram_tensor("yt0", [P, 2 * FPP], f16, kind="ExternalInput").ap()
    out = nc.dram_tensor("out", [1, 3], f32, kind="ExternalOutput").ap()

    offs = [0]
    for FD in FDS:
        offs.append(offs[-1] + FD)
    toffs = [0]
    for FD in TDS:
        toffs.append(toffs[-1] + FD)

    with tile.TileContext(nc) as tc:
        with (
            tc.tile_pool(name="io", bufs=1) as io,
            tc.tile_pool(name="mid", bufs=2) as mid,
            tc.tile_pool(name="accs", bufs=1) as accs,
            tc.tile_pool(name="psum", bufs=1, space=bass.MemorySpace.PSUM) as psum,
        ):
            acc_p = accs.tile([P, NT], f32)   # per-chunk sum(d^2/v)
            acc_r = accs.tile([P, len(RED_SLICES)], f32)  # bit-sum partials
            acc_l = accs.tile([P, 1], f32)    # exact ln partial (tail range)
            outt = accs.tile([P, 3], f32)
            outs = accs.tile([1, 3], f32)
            pacc = psum.tile([1, 3], f32)
            abias = accs.tile([P, 1], f32)
            zbias = accs.tile([P, 1], f32)
            ones = accs.tile([P, 1], f32)
            nc.vector.memset(abias[:], a)
            nc.vector.memset(zbias[:], 0.0)
            nc.vector.memset(ones[:], 1.0)

            # big contiguous t1 / u / v planes; DMA + ACT use subtile ranges
            t1t = io.tile([P, FPP], f16, tag="t1t")
            ut = io.tile([P, FPP], f32, tag="ut")
            vt = io.tile([P, FPP], f16, tag="vt")
            vt_i16 = vt.bitcast(i16)

            yt0_t = {}

            def dma_t1(eng, k):
                eng.dma_start(
                    t1t[:, toffs[k] : toffs[k + 1]], t1[:, toffs[k] : toffs[k + 1]]
                )

            def dma_yt0(eng, i):
                yt0_t[i] = io.tile(
                    [P, 2 * FDS[i]], f16, tag=f"yt0_{i}", name=f"tyt0_{i}"
                )
                eng.dma_start(yt0_t[i][:], yt0[:, 2 * offs[i] : 2 * offs[i + 1]])

            # All descriptors on one engine (sync): program order == queue
            # FIFO order == arrival order, no cross-engine races.  t1 leads
            # (it gates the serial ACT chain), yt0 interleaves behind.
            dma_t1(nc.sync, 0)
            dma_t1(nc.sync, 1)
            dma_t1(nc.sync, 2)
            dma_yt0(nc.sync, 0)
            dma_t1(nc.sync, 3)
            for i in range(1, NT):
                dma_yt0(nc.sync, i)

            # Tiny dependency-free first activation: the one-time
            # ACT_TABLE_LOAD bacc inserts before the first ACT instruction
            # executes at boot instead of after t1_0 lands.
            warm = accs.tile([P, 1], f32)
            nc.scalar.activation(warm[:], zbias[:], Act.Exp, bias=zbias[:, 0:1])

            # Per-grain pipeline: ACT exp+ln; on the DVE the sub for chunk
            # i+1 is emitted BEFORE sqdiv_i, so subs execute in the DVE's
            # wait-for-ln gaps (their yt0 data lands well ahead) instead of
            # stacking up behind the sqdiv chain at the end.
            red_next = 0
            d_t = {}

            def emit_sub(j):
                tyt = yt0_t.pop(j)
                d_t[j] = mid.tile([P, FDS[j]], f16, tag="d", bufs=3, name=f"d_{j}")
                nc.vector.tensor_sub(
                    d_t[j][:], tyt[:, 0 : FDS[j]], tyt[:, FDS[j] : 2 * FDS[j]]
                )

            emit_sub(0)
            for i in range(NT):
                FD = FDS[i]
                sl = slice(offs[i], offs[i + 1])

                nc.scalar.activation(ut[:, sl], t1t[:, sl], Act.Exp, bias=zbias[:, 0:1])
                nc.scalar.activation(
                    vt[:, sl], ut[:, sl], Act.Ln, bias=abias[:, 0:1], scale=a
                )
                if i + 1 < NT:
                    emit_sub(i + 1)
                scr = mid.tile([P, FD], f16, tag="scr")
                nc.vector._custom_dve(
                    SQDIV_RED,
                    out=scr[:],
                    in0=d_t.pop(i)[:],
                    in1=vt[:, sl],
                    s0=RC0,
                    s1=RC1,
                    accum_out=acc_p[:, i : i + 1],
                )
                while red_next < len(RED_SLICES) and RED_SLICES[red_next][1] <= offs[i + 1]:
                    lo, hi = RED_SLICES[red_next]
                    nc.vector.reduce_sum(
                        acc_r[:, red_next : red_next + 1],
                        vt_i16[:, lo:hi],
                        axis=mybir.AxisListType.X,
                    )
                    red_next += 1

            # Exact sum(ln v) over the tail range as ONE ACT instruction at
            # the very end of the scalar stream: the DVE drains its
            # remaining sqdiv/bit-sum backlog underneath it, so the two
            # engines finish together instead of DVE trailing by ~3us.
            lsl = slice(RED_END, FPP)
            nc.scalar.activation(
                ut[:, lsl],
                vt[:, lsl],
                Act.Ln,
                bias=zbias[:, 0:1],
                accum_out=acc_l[:, 0:1],
            )

            # final folds: [sum d^2/v, bit-sum, exact-ln sum] per partition,
            # PE ones-matmul folds the 128 partitions -> [1, 3]
            nc.vector.reduce_sum(outt[:, 0:1], acc_p[:], axis=mybir.AxisListType.X)
            nc.vector.reduce_sum(outt[:, 1:2], acc_r[:], axis=mybir.AxisListType.X)
            nc.vector.tensor_copy(outt[:, 2:3], acc_l[:, 0:1])
            nc.tensor.matmul(pacc[:, :], ones[:, 0:1], outt[:, 0:3])
            nc.vector.tensor_copy(outs[:, :], pacc[:, :])
            nc.sync.dma_start(out[:], outs[:])

    nc.compile()
    return nc


def kernel(tensor, y_target, noise_unconstrained):
    global LAST_RESULT
    noise = np.float64(np.asarray(noise_unconstrained))
    c = np.log1p(np.exp(-abs(noise))) + max(noise, 0.0) + JITTER  # softplus+jitter
    a = float(np.exp(c))

    nc = _BUILD_CACHE.get(a)
    if nc is None:
        nc = _build(a)
        _BUILD_CACHE[a] = nc

    tensor = np.asarray(tensor, dtype=np.float32)
    y_target = np.asarray(y_target, dtype=np.float32)

    offs = [0]
    for FD in FDS:
        offs.append(offs[-1] + FD)

    in_maps = []
    for k in range(NCORES):
        sh = tensor[k * ROWS : (k + 1) * ROWS]          # (64, 16384, 2)
        t1p = sh[:, :, 1].reshape(P, FPP).astype(np.float16)
        yp = y_target[k * ROWS : (k + 1) * ROWS, :, 0].reshape(P, FPP).astype(
            np.float16
        )
        t0p = sh[:, :, 0].reshape(P, FPP).astype(np.float16)
        yt0p = np.empty((P, 2 * FPP), dtype=np.float16)
        for i in range(NT):
            lo, hi = offs[i], offs[i + 1]
            yt0p[:, 2 * lo : lo + hi] = yp[:, lo:hi]
            yt0p[:, lo + hi : 2 * hi] = t0p[:, lo:hi]
        in_maps.append({"t1": t1p, "yt0": yt0p})

    trace = os.environ.get("BASS_KERNEL_PROFILE", "0") == "1"
    res = bass_utils.run_bass_kernel_spmd(
        nc, in_maps, list(range(NCORES)), trace=trace
    )
    LAST_RESULT = res

    s_p = np.float64(0.0)    # sum d^2 / y_var
    s_i = np.float64(0.0)    # sum of fp16 bit patterns of v (early grains)
    s_ln = np.float64(0.0)   # exact sum ln v (late grains)
    for k in range(NCORES):
        o = np.asarray(res.results[k]["out"], dtype=np.float64)
        s_p += o[0, 0]
        s_i += o[0, 1]
        s_ln += o[0, 2]
    offs = [0]
    for FD in FDS:
        offs.append(offs[-1] + FD)
    n_bits = np.float64(RED_END * P * NCORES)
    n_tot = np.float64(B) * np.float64(T)
    s_ln += LN2 * (s_i / 1024.0 + n_bits * (MITCHELL_SIGMA - 15.0))
    total = n_tot * np.float64(LOG_2PI) + s_ln + s_p
    return np.array(-0.5 * total / B, dtype=np.float32)
